# revision 4
# baseline (speedup 1.0000x reference)
"""CGGAT (ALIGNN-style GNN) for Trainium2, 8 NeuronCores.

Strategy (v1): all dense math (matmuls, layernorms, activations, residuals)
runs on the 8 NeuronCores via two reusable Bass/Tile kernels, row-sharded
8 ways.  Host (numpy) does index-glue only-ish: gathers, segment-sums,
RBF featurization and the tiny first MLP stages + final pooling.

Device kernels (compiled once, reused for every layer):
  K1 "conv front": per row r:
      z  = G1@Wsg + G2@Wdg + E3@Weg + bg
      sig= sigmoid(z);  gu = G1@Wsu + bsu;  m = sig*gu
      eup= E3 + silu(LN(z)*gn2+bn2)
      outputs msig=[m|sig] and eup
  K2 "x update / mlp": per row r:
      y = RESID + silu(LN(X@W + b + AGG)*g + be)
"""
import sys, os
sys.path.insert(0, '/opt/trn_rl_repo')
import numpy as np
import ml_dtypes

BF = ml_dtypes.bfloat16
P = 128
RMAX = 104448          # rows per core per launch (816 tiles of 128)
NT = RMAX // P
N_CORES = 8

_cache = {}


def _build_kernels():
    if 'k1' in _cache:
        return
    import concourse.bass as bass
    import concourse.tile as tile
    import concourse.bacc as bacc
    from concourse import mybir
    from concourse.masks import make_identity
    BF16 = mybir.dt.bfloat16
    F32 = mybir.dt.float32
    AF = mybir.ActivationFunctionType
    OP = mybir.AluOpType
    AX = mybir.AxisListType

    def ln_silu(nc, pool, z_sb, g_t, b_t, eps_t, out_dtype=BF16):
        """row-wise LN over 128 feats + *g+b + silu; returns sbuf tile."""
        mean = pool.tile([P, 1], F32, tag='mean')
        nc.vector.reduce_sum(mean[:], z_sb[:], axis=AX.X)
        nc.scalar.mul(mean[:], mean[:], 1.0 / 128.0)
        xc = pool.tile([P, P], F32, tag='xc')
        nc.vector.tensor_tensor(out=xc[:], in0=z_sb[:], in1=mean[:].to_broadcast([P, P]),
                                op=OP.subtract)
        sq = pool.tile([P, P], F32, tag='sq')
        nc.vector.tensor_tensor(out=sq[:], in0=xc[:], in1=xc[:], op=OP.mult)
        var = pool.tile([P, 1], F32, tag='var')
        nc.vector.reduce_sum(var[:], sq[:], axis=AX.X)
        nc.scalar.mul(var[:], var[:], 1.0 / 128.0)
        rstd = pool.tile([P, 1], F32, tag='rstd')
        nc.vector.tensor_tensor(out=var[:], in0=var[:], in1=eps_t[:], op=OP.add)
        nc.scalar.activation(rstd[:], var[:], AF.Sqrt)
        nc.vector.reciprocal(rstd[:], rstd[:])
        nc.vector.tensor_tensor(out=xc[:], in0=xc[:], in1=rstd[:].to_broadcast([P, P]),
                                op=OP.mult)
        nc.vector.tensor_tensor(out=xc[:], in0=xc[:], in1=g_t[:], op=OP.mult)
        nc.vector.tensor_tensor(out=xc[:], in0=xc[:], in1=b_t[:], op=OP.add)
        sil = pool.tile([P, P], out_dtype, tag='sil')
        nc.scalar.activation(sil[:], xc[:], AF.Silu)
        return sil

    # ---------------- K1 ----------------
    nc = bacc.Bacc('TRN2', target_bir_lowering=False)
    g1_d = nc.dram_tensor('g1', [RMAX, P], BF16, kind='ExternalInput')
    g2_d = nc.dram_tensor('g2', [RMAX, P], BF16, kind='ExternalInput')
    e3_d = nc.dram_tensor('e3', [RMAX, P], BF16, kind='ExternalInput')
    w4_d = nc.dram_tensor('w4', [P, 4 * P], BF16, kind='ExternalInput')   # Wsg|Wdg|Weg|Wsu
    cv_d = nc.dram_tensor('cv', [P, 4 * P], mybir.dt.float32, kind='ExternalInput')  # bg|bsu|gn2|bn2
    ms_d = nc.dram_tensor('msig', [RMAX, 2 * P], BF16, kind='ExternalOutput')
    eu_d = nc.dram_tensor('eup', [RMAX, P], BF16, kind='ExternalOutput')
    with tile.TileContext(nc) as tc:
        with (tc.tile_pool(name='const', bufs=1) as cp,
              tc.tile_pool(name='io', bufs=3) as iop,
              tc.tile_pool(name='work', bufs=2) as wp,
              tc.tile_pool(name='ps', bufs=2, space='PSUM') as psp,
              tc.tile_pool(name='pst', bufs=2, space='PSUM') as pstp):
            w4 = cp.tile([P, 4 * P], BF16)
            nc.sync.dma_start(w4[:], w4_d[:])
            cv = cp.tile([P, 4 * P], mybir.dt.float32)
            nc.sync.dma_start(cv[:], cv_d[:])
            ident = cp.tile([P, P], BF16)
            make_identity(nc, ident[:])
            eps_t = cp.tile([P, 1], mybir.dt.float32)
            nc.vector.memset(eps_t[:], 1e-5)

            def body(i):
                g1 = iop.tile([P, P], BF16, tag='g1')
                g2 = iop.tile([P, P], BF16, tag='g2')
                e3 = iop.tile([P, P], BF16, tag='e3')
                nc.sync.dma_start(g1[:], g1_d[bass.ts(i, P), :])
                nc.sync.dma_start(g2[:], g2_d[bass.ts(i, P), :])
                nc.sync.dma_start(e3[:], e3_d[bass.ts(i, P), :])
                fs = []
                for src, tg in [(g1, 'f1'), (g2, 'f2'), (e3, 'f3')]:
                    tp = pstp.tile([P, P], BF16, tag='tp')
                    nc.tensor.transpose(out=tp[:], in_=src[:], identity=ident[:])
                    f = wp.tile([P, P], BF16, tag=tg)
                    nc.vector.tensor_copy(f[:], tp[:])
                    fs.append(f)
                g1f, g2f, e3f = fs
                zg = psp.tile([P, 2 * P], mybir.dt.float32, tag='zg')
                nc.tensor.matmul(zg[:, 0:P], lhsT=g1f[:], rhs=w4[:, 0:P], start=True, stop=False)
                nc.tensor.matmul(zg[:, P:2 * P], lhsT=g1f[:], rhs=w4[:, 3 * P:4 * P], start=True, stop=True)
                nc.tensor.matmul(zg[:, 0:P], lhsT=g2f[:], rhs=w4[:, P:2 * P], start=False, stop=False)
                nc.tensor.matmul(zg[:, 0:P], lhsT=e3f[:], rhs=w4[:, 2 * P:3 * P], start=False, stop=True)
                z_sb = wp.tile([P, P], mybir.dt.float32, tag='z')
                nc.vector.tensor_tensor(out=z_sb[:], in0=zg[:, 0:P], in1=cv[:, 0:P], op=OP.add)
                sig = wp.tile([P, P], BF16, tag='sig')
                nc.scalar.activation(sig[:], z_sb[:], AF.Sigmoid)
                gu = wp.tile([P, P], mybir.dt.float32, tag='gu')
                nc.vector.tensor_tensor(out=gu[:], in0=zg[:, P:2 * P], in1=cv[:, P:2 * P], op=OP.add)
                mm = wp.tile([P, P], BF16, tag='m')
                nc.vector.tensor_tensor(out=mm[:], in0=gu[:], in1=sig[:], op=OP.mult)
                nc.sync.dma_start(ms_d[bass.ts(i, P), 0:P], mm[:])
                nc.sync.dma_start(ms_d[bass.ts(i, P), P:2 * P], sig[:])
                sil = ln_silu(nc, wp, z_sb, cv[:, 2 * P:3 * P], cv[:, 3 * P:4 * P], eps_t)
                eup = wp.tile([P, P], BF16, tag='eup')
                nc.vector.tensor_tensor(out=eup[:], in0=e3[:], in1=sil[:], op=OP.add)
                nc.sync.dma_start(eu_d[bass.ts(i, P), :], eup[:])

            tc.For_i_unrolled(0, NT, 1, body, max_unroll=4)
    nc.compile()
    _cache['k1'] = nc

    # ---------------- K2 ----------------
    nc = bacc.Bacc('TRN2', target_bir_lowering=False)
    x_d = nc.dram_tensor('x', [RMAX, P], BF16, kind='ExternalInput')
    ag_d = nc.dram_tensor('agg', [RMAX, P], BF16, kind='ExternalInput')
    rs_d = nc.dram_tensor('resid', [RMAX, P], BF16, kind='ExternalInput')
    w_d = nc.dram_tensor('w', [P, P], BF16, kind='ExternalInput')
    cv2_d = nc.dram_tensor('cv2', [P, 3 * P], mybir.dt.float32, kind='ExternalInput')  # b|g|be
    y_d = nc.dram_tensor('y', [RMAX, P], BF16, kind='ExternalOutput')
    with tile.TileContext(nc) as tc:
        with (tc.tile_pool(name='const', bufs=1) as cp,
              tc.tile_pool(name='io', bufs=3) as iop,
              tc.tile_pool(name='work', bufs=2) as wp,
              tc.tile_pool(name='ps', bufs=2, space='PSUM') as psp,
              tc.tile_pool(name='pst', bufs=2, space='PSUM') as pstp):
            w = cp.tile([P, P], BF16)
            nc.sync.dma_start(w[:], w_d[:])
            cv2 = cp.tile([P, 3 * P], mybir.dt.float32)
            nc.sync.dma_start(cv2[:], cv2_d[:])
            ident = cp.tile([P, P], BF16)
            make_identity(nc, ident[:])
            eps_t = cp.tile([P, 1], mybir.dt.float32)
            nc.vector.memset(eps_t[:], 1e-5)

            def body2(i):
                x = iop.tile([P, P], BF16, tag='x')
                ag = iop.tile([P, P], BF16, tag='ag')
                rs = iop.tile([P, P], BF16, tag='rs')
                nc.sync.dma_start(x[:], x_d[bass.ts(i, P), :])
                nc.sync.dma_start(ag[:], ag_d[bass.ts(i, P), :])
                nc.sync.dma_start(rs[:], rs_d[bass.ts(i, P), :])
                tp = pstp.tile([P, P], BF16, tag='tp')
                nc.tensor.transpose(out=tp[:], in_=x[:], identity=ident[:])
                xf = wp.tile([P, P], BF16, tag='xf')
                nc.vector.tensor_copy(xf[:], tp[:])
                zp = psp.tile([P, P], mybir.dt.float32, tag='zp')
                nc.tensor.matmul(zp[:], lhsT=xf[:], rhs=w[:], start=True, stop=True)
                z_sb = wp.tile([P, P], mybir.dt.float32, tag='z')
                nc.vector.tensor_tensor(out=z_sb[:], in0=zp[:], in1=cv2[:, 0:P], op=OP.add)
                nc.vector.tensor_tensor(out=z_sb[:], in0=z_sb[:], in1=ag[:], op=OP.add)
                sil = ln_silu(nc, wp, z_sb, cv2[:, P:2 * P], cv2[:, 2 * P:3 * P], eps_t)
                y = wp.tile([P, P], BF16, tag='y')
                nc.vector.tensor_tensor(out=y[:], in0=rs[:], in1=sil[:], op=OP.add)
                nc.sync.dma_start(y_d[bass.ts(i, P), :], y[:])

            tc.For_i_unrolled(0, NT, 1, body2, max_unroll=4)
    nc.compile()
    _cache['k2'] = nc


def _pad_rows(x, n):
    out = np.zeros((n,) + x.shape[1:], x.dtype)
    out[:len(x)] = x
    return out


def _shard(x_full):
    """split rows into 8 shards padded to RMAX -> list of [RMAX, D]."""
    n = len(x_full)
    per = -(-n // N_CORES)
    shards = []
    for c in range(N_CORES):
        shards.append(_pad_rows(x_full[c * per:(c + 1) * per], RMAX))
    return shards, per


def _run(nc, in_maps):
    from concourse import bass_utils
    res = bass_utils.run_bass_kernel_spmd(nc, in_maps, core_ids=list(range(N_CORES)))
    return res.results


def _k1(g1, g2, e3, wsg, wdg, weg, wsu, bg, bsu, gn2, bn2):
    """full arrays [R,128] -> msig [R,256], eup [R,128] (f32 views)."""
    _build_kernels()
    R = len(g1)
    s1, per = _shard(g1.astype(BF))
    s2, _ = _shard(g2.astype(BF))
    s3, _ = _shard(e3.astype(BF))
    w4 = np.concatenate([wsg, wdg, weg, wsu], axis=1).astype(BF)
    rep = lambda v: np.tile(np.asarray(v, np.float32)[None, :], (P, 1))
    cv = np.concatenate([rep(bg), rep(bsu), rep(gn2), rep(bn2)], axis=1)
    maps = [{'g1': s1[c], 'g2': s2[c], 'e3': s3[c], 'w4': w4, 'cv': cv}
            for c in range(N_CORES)]
    res = _run(_cache['k1'], maps)
    ms = np.concatenate([res[c]['msig'][:per] for c in range(N_CORES)], 0)[:R]
    eu = np.concatenate([res[c]['eup'][:per] for c in range(N_CORES)], 0)[:R]
    return ms.astype(np.float32), eu.astype(np.float32)


def _k2(x, agg, resid, w, b, g, be):
    _build_kernels()
    R = len(x)
    sx, per = _shard(x.astype(BF))
    sa, _ = _shard(agg.astype(BF))
    sr, _ = _shard(resid.astype(BF))
    rep = lambda v: np.tile(np.asarray(v, np.float32)[None, :], (P, 1))
    cv2 = np.concatenate([rep(b), rep(g), rep(be)], axis=1)
    maps = [{'x': sx[c], 'agg': sa[c], 'resid': sr[c],
             'w': np.asarray(w, np.float32).astype(BF), 'cv2': cv2}
            for c in range(N_CORES)]
    res = _run(_cache['k2'], maps)
    y = np.concatenate([res[c]['y'][:per] for c in range(N_CORES)], 0)[:R]
    return y.astype(np.float32)


def _pad_w(w):
    """pad [din, dout] weight to [128, 128]"""
    out = np.zeros((P, P), np.float32)
    w = np.asarray(w, np.float32)
    out[:w.shape[0], :w.shape[1]] = w
    return out


def _egat_conv(src, dst, x, e, p, n_dst):
    g1 = x[src]
    g2 = x[dst]
    ms, e_new = _k1(g1, g2, e, p['Wsg'], p['Wdg'], p['Weg'], p['Wsu'],
                    p['bg'], p['bsu'], p['gn2'], p['bn2'])
    m, sig = ms[:, :P], ms[:, P:]
    order = np.argsort(dst, kind='stable')
    ds_ = dst[order]
    starts = np.searchsorted(ds_, np.arange(n_dst))
    ends = np.searchsorted(ds_, np.arange(n_dst) + 1)
    cs_m = np.concatenate([np.zeros((1, P)), np.cumsum(m[order].astype(np.float64), 0)], 0)
    cs_s = np.concatenate([np.zeros((1, P)), np.cumsum(sig[order].astype(np.float64), 0)], 0)
    num = (cs_m[ends] - cs_m[starts]).astype(np.float32)
    den = (cs_s[ends] - cs_s[starts]).astype(np.float32)
    agg = num / (den + 1e-6)
    x_new = _k2(x, agg, x, p['Wdu'], p['bdu'], p['gn1'], p['bn1'])
    return x_new, e_new


def _mlp_host_then_dev(feat, p1, p2):
    """two-stage mlp: host does stage1 (small dims), device stage2."""
    h = feat @ np.asarray(p1['W'], np.float32) + np.asarray(p1['b'], np.float32)
    mu = h.mean(-1, keepdims=True)
    v = ((h - mu) ** 2).mean(-1, keepdims=True)
    h = (h - mu) / np.sqrt(v + 1e-5) * np.asarray(p1['g'], np.float32) + np.asarray(p1['be'], np.float32)
    h = h / (1.0 + np.exp(-h))
    hp = np.zeros((len(h), P), np.float32)
    hp[:, :h.shape[1]] = h
    return _k2(hp, np.zeros_like(hp), np.zeros_like(hp),
               _pad_w(p2['W']), p2['b'], p2['g'], p2['be'])


def _rbf(x, vmin, vmax, bins):
    centers = np.linspace(vmin, vmax, bins).astype(np.float32)
    gamma = 1.0 / (centers[1] - centers[0])
    return np.exp(-gamma * (x[..., None] - centers) ** 2).astype(np.float32)


def kernel(params, atom_features, r, angle_h, src, dst, esrc, edst, node2graph):
    t_np = lambda v: np.asarray(v)
    atom_features = t_np(atom_features).astype(np.float32)
    r = t_np(r).astype(np.float32)
    angle_h = t_np(angle_h).astype(np.float32)
    src = t_np(src).astype(np.int64)
    dst = t_np(dst).astype(np.int64)
    esrc = t_np(esrc).astype(np.int64)
    edst = t_np(edst).astype(np.int64)
    node2graph = t_np(node2graph).astype(np.int64)
    n_nodes = atom_features.shape[0]
    n_edges = src.shape[0]
    n_graphs = 32

    pp = {k: {kk: np.asarray(vv, np.float32) for kk, vv in v.items()}
          if isinstance(v, dict) else v for k, v in params.items()}

    # embeddings
    a = _mlp_host_then_dev(_rbf(angle_h, -1.0, 1.0, 40), pp['ang1'], pp['ang2'])
    bond = np.linalg.norm(r, axis=1)
    e = _mlp_host_then_dev(_rbf(bond, 0.0, 8.0, 80), pp['edge1'], pp['edge2'])
    # atom: single mlp 92->128 on device (stage2-only path with padded input)
    ap_ = np.zeros((n_nodes, P), np.float32)
    ap_[:, :atom_features.shape[1]] = atom_features
    atom_p = pp['atom']
    v = _k2(ap_, np.zeros_like(ap_), np.zeros_like(ap_),
            _pad_w(atom_p['W']), atom_p['b'], atom_p['g'], atom_p['be'])

    node_convs = [params['node_convs'][i] for i in range(3)]
    edge_convs = [params['edge_convs'][i] for i in range(3)]
    for i in range(3):
        ncv = {k: np.asarray(vv, np.float32) for k, vv in node_convs[i].items()}
        ecv = {k: np.asarray(vv, np.float32) for k, vv in edge_convs[i].items()}
        v, e = _egat_conv(src, dst, v, e, ncv, n_nodes)
        e, a = _egat_conv(esrc, edst, e, a, ecv, n_edges)
    fin = {k: np.asarray(vv, np.float32) for k, vv in params['final'].items()}
    v, e = _egat_conv(src, dst, v, e, fin, n_nodes)

    sums = np.zeros((n_graphs, P), np.float32)
    np.add.at(sums, node2graph, v)
    counts = np.bincount(node2graph, minlength=n_graphs).astype(np.float32)
    h = sums / np.maximum(counts, 1.0)[:, None]
    out = h @ np.asarray(pp['fc']['W'], np.float32) + np.asarray(pp['fc']['b'], np.float32)
    return np.squeeze(out).astype(np.float32)
